# revision 34
# baseline (speedup 1.0000x reference)
"""Multi-head causal attention (B=4, T=2048, C=1024, H=16) on 8 TRN2 NeuronCores.

Sharding: data-parallel over batch (4) x tensor-parallel over heads (2 groups
of 8 heads). Core c handles batch c%4, head-group c//4. Each core:
  - QKV projection in transposed layout (float32r matmuls, full PE rate):
    Q^T/K^T/V^T [m, t] tiles, bias folded into the psum->SBUF copy (bf16 out).
  - V^T -> V via PE transposes.
  - Causal flash-style attention per head-pair (2 heads packed per matmul via
    row/col tile_position): S^T = K^T.T @ Q^T, E = exp(S^T) on ScalarE,
    input mask applied on diagonal-band tiles, AV and denominator (ones-matmul)
    accumulated over key tiles; no max-subtraction needed (logits are small:
    softmax is shift-invariant, fp32 exp cannot overflow here).
  - Row-parallel output projection producing a partial [T, C] sum; host adds
    the two head-group partials (out bias is added on head-group-0 cores only).
"""

import os
import sys

sys.path.insert(0, "/opt/trn_rl_repo")

import numpy as np
import ml_dtypes

import concourse.bacc as bacc
import concourse.tile as tile
from concourse import mybir
from concourse.bass_utils import run_bass_kernel_spmd
from concourse.masks import make_identity

B, T, C, H, D = 4, 2048, 1024, 16, 64
HPC = 8          # heads per core
PAIRS = HPC // 2
CT = C // 128    # 8 contraction tiles for the projections
MT = 12          # qkv m-tiles per core (4 pairs x {q,k,v})
NQB = T // 512   # 4 query blocks of 512
NKT = T // 128   # 16 key tiles of 128

F32 = mybir.dt.float32
F32R = mybir.dt.float32r
BF16 = mybir.dt.bfloat16

LAST_RESULT = None  # stashed BassKernelResults for test harnesses


def build():
    nc = bacc.Bacc("TRN2", target_bir_lowering=False)

    xT = nc.dram_tensor("xT", [C, T], BF16, kind="ExternalInput")
    wqkvT = nc.dram_tensor("wqkvT", [C, MT * 128], BF16, kind="ExternalInput")
    bqkv = nc.dram_tensor("bqkv", [128, MT], F32, kind="ExternalInput")
    woT = nc.dram_tensor("woT", [512, C], BF16, kind="ExternalInput")
    bo = nc.dram_tensor("bo", [128, C], F32, kind="ExternalInput")
    maskband = nc.dram_tensor("maskband", [128, NKT, 512], BF16, kind="ExternalInput")
    out = nc.dram_tensor("out", [T, C], F32, kind="ExternalOutput")

    with tile.TileContext(nc) as tc:
        with tc.tile_pool(name="persist", bufs=1) as pp, \
             tc.tile_pool(name="stream", bufs=2) as sp, \
             tc.tile_pool(name="pss", bufs=3, space="PSUM") as pss, \
             tc.tile_pool(name="psav", bufs=1, space="PSUM") as psav, \
             tc.tile_pool(name="psden", bufs=1, space="PSUM") as psden:

            # -------- weight DMAs for the first m-tiles go out first --------
            wm_tiles = {}

            def fetch_wm(mt):
                wm = sp.tile([128, CT, 128], BF16, tag="wm", bufs=4, name=f"wm{mt}")
                nc.sync.dma_start(
                    out=wm,
                    in_=wqkvT[:, mt * 128:(mt + 1) * 128]
                    .rearrange("(n p) m -> p n m", p=128))
                wm_tiles[mt] = wm

            fetch_wm(0)
            xt_sb = []
            for ct in range(CT):
                t_ = pp.tile([128, T], BF16, tag="xt", bufs=CT, name=f"xt{ct}")
                # quarter DMA first so the first matmuls can start early
                nc.sync.dma_start(out=t_[:, 0:512],
                                  in_=xT[ct * 128:(ct + 1) * 128, 0:512])
                xt_sb.append(t_)
            fetch_wm(1)
            for ct in range(CT):
                nc.sync.dma_start(out=xt_sb[ct][:, 512:1024],
                                  in_=xT[ct * 128:(ct + 1) * 128, 512:1024])
            fetch_wm(2)
            for ct in range(CT):
                nc.sync.dma_start(out=xt_sb[ct][:, 1024:2048],
                                  in_=xT[ct * 128:(ct + 1) * 128, 1024:2048])

            # ---------------- other constants / persistent inputs ----------------
            ones_sb = pp.tile([128, 64], BF16, tag="ones", name="ones")
            nc.vector.memset(ones_sb, 1.0)
            ident = pp.tile([128, 128], BF16, tag="ident", name="ident")
            make_identity(nc, ident)
            bqkv_sb = pp.tile([128, MT], F32, tag="bqkv", name="bqkv_sb")
            nc.sync.dma_start(out=bqkv_sb, in_=bqkv[:, :])
            bo_sb = pp.tile([128, C], F32, tag="bo", name="bo_sb")
            nc.sync.dma_start(out=bo_sb, in_=bo[:, :])
            mask_sb = pp.tile([128, NKT, 512], BF16, tag="mask", name="mask_sb")
            nc.sync.dma_start(out=mask_sb, in_=maskband[:, :, :])
            wo_sb = []
            for it in range(4):
                w = pp.tile([128, C], BF16, tag="wo", bufs=4, name=f"wo{it}")
                nc.sync.dma_start(out=w, in_=woT[it * 128:(it + 1) * 128, :])
                wo_sb.append(w)

            QT = [pp.tile([128, T], BF16, tag="qt", bufs=4, name=f"qt{p}") for p in range(4)]
            KT = [pp.tile([128, T], BF16, tag="kt", bufs=4, name=f"kt{p}") for p in range(4)]
            V = [pp.tile([128, T], BF16, tag="v", bufs=4, name=f"v{p}") for p in range(4)]
            AT = [pp.tile([128, T], BF16, tag="at", bufs=4, name=f"at{p}") for p in range(4)]

            # ---------------- QKV projection (transposed output) ----------------
            def qkv_mtile(p, j):
                mt = p * 3 + j
                if mt + 1 < MT and (mt + 1) not in wm_tiles:
                    fetch_wm(mt + 1)   # prefetch next weight slice
                wm = wm_tiles.pop(mt)
                dst = (QT[p], KT[p], None)[j]
                if j == 2:
                    dst = pp.tile([128, T], BF16, tag="vt", bufs=2, name=f"vt{p}")
                if mt == 0:  # fine-grained start using the idle av/den banks
                    for tch in range(4):
                        pool = (psav, psden)[tch % 2]
                        ps = pool.tile([128, 512], F32, tag=("av", "den")[tch % 2],
                                       name=f"qkv_ps0_{tch}")
                        for ct in range(CT):
                            nc.tensor.matmul(
                                ps, wm[:, ct, :], xt_sb[ct][:, tch * 512:(tch + 1) * 512],
                                start=(ct == 0), stop=(ct == CT - 1))
                        nc.vector.tensor_scalar_add(
                            dst[:, tch * 512:(tch + 1) * 512], ps, bqkv_sb[:, mt:mt + 1])
                    return
                for tch in range(2):
                    ps = pss.tile([128, 1024], F32, tag="pss", name=f"qkv_ps{mt}_{tch}")
                    for ct in range(CT):
                        for hf in range(2):
                            t0 = tch * 1024 + hf * 512
                            nc.tensor.matmul(
                                ps[:, hf * 512:(hf + 1) * 512],
                                wm[:, ct, :], xt_sb[ct][:, t0:t0 + 512],
                                start=(ct == 0), stop=(ct == CT - 1))
                    nc.vector.tensor_scalar_add(
                        dst[:, tch * 1024:(tch + 1) * 1024], ps, bqkv_sb[:, mt:mt + 1])
                if j == 2:  # V^T -> V natural layout via PE transposes
                    for g in range(4):
                        pst = psav.tile([128, 512], BF16, tag="av", name=f"vtr{p}_{g}")
                        for c4 in range(4):
                            k = g * 4 + c4
                            nc.tensor.transpose(
                                pst[:, c4 * 128:(c4 + 1) * 128],
                                dst[:, k * 128:(k + 1) * 128], ident)
                        nc.vector.tensor_copy(V[p][:, g * 512:(g + 1) * 512], pst)

            # ---------------- attention unit + out-projection ----------------
            def outproj(tt):
                po = pss.tile([128, 1024], F32, tag="pss", name=f"op{tt}")
                for oc in range(2):
                    for it in range(4):
                        nc.tensor.matmul(
                            po[:, oc * 512:(oc + 1) * 512],
                            AT[it][:, tt * 128:(tt + 1) * 128],
                            wo_sb[it][:, oc * 512:(oc + 1) * 512],
                            start=(it == 0), stop=(it == 3))
                o = sp.tile([128, 1024], F32, tag="o", bufs=4, name=f"o{tt}")
                nc.vector.tensor_add(o, po, bo_sb)
                nc.sync.dma_start(out=out[tt * 128:(tt + 1) * 128, :], in_=o)

            def attn_unit(qb, p):
                nkt = qb * 4 + 4
                q0 = qb * 512
                av = psav.tile([128, 512], F32, tag="av", name=f"av{qb}_{p}")
                den = psden.tile([128, 512], F32, tag="den", name=f"den{qb}_{p}")

                def flush(prev):
                    k, e, off, w = prev
                    st, sp_ = (k == 0), (k == nkt - 1)
                    nc.tensor.matmul(av[0:64, off:off + w],
                                     V[p][:, k * 128:k * 128 + 64], e[:, 0:w],
                                     start=st, stop=sp_, skip_group_check=True)
                    nc.tensor.matmul(av[64:128, off:off + w],
                                     V[p][:, k * 128 + 64:(k + 1) * 128], e[:, 512:512 + w],
                                     start=st, stop=sp_, skip_group_check=True)
                    nc.tensor.matmul(den[0:64, off:off + w], ones_sb, e[:, 0:w],
                                     start=st, stop=sp_, skip_group_check=True)
                    nc.tensor.matmul(den[64:128, off:off + w], ones_sb, e[:, 512:512 + w],
                                     start=st, stop=sp_, skip_group_check=True)

                prev = None
                for k in range(nkt):
                    koff = k - qb * 4
                    # causal: kj-tile k only reaches queries qi >= k*128
                    off = max(koff, 0) * 128
                    w = 512 - off
                    qa = q0 + off
                    ss = pss.tile([128, 1024], F32, tag="pss", name=f"ss{qb}_{p}_{k}")
                    nc.tensor.matmul(ss[:, 0:w], KT[p][0:64, k * 128:(k + 1) * 128],
                                     QT[p][0:64, qa:qa + w], start=True, stop=True)
                    nc.tensor.matmul(ss[:, 512:512 + w], KT[p][64:128, k * 128:(k + 1) * 128],
                                     QT[p][64:128, qa:qa + w], start=True, stop=True)
                    e = sp.tile([128, 2, 512], BF16, tag="e", bufs=12, name=f"e{qb}_{p}_{k}")
                    nc.scalar.activation(
                        e[:, :, 0:w],
                        ss[:, :].rearrange("a (two n) -> a two n", two=2)[:, :, 0:w],
                        mybir.ActivationFunctionType.Exp)
                    ef = e.rearrange("a two n -> a (two n)")
                    if koff >= 0:
                        # apply the input mask on the leading 128-wide block only:
                        # beyond it every query index exceeds all keys of this tile
                        # (causal tril), so the mask there is all-ones
                        nc.vector.tensor_mul(ef[:, 0:128], ef[:, 0:128],
                                             mask_sb[:, k, off:off + 128])
                        nc.vector.tensor_mul(ef[:, 512:640], ef[:, 512:640],
                                             mask_sb[:, k, off:off + 128])
                    if prev is not None:
                        flush(prev)
                    prev = (k, ef, off, w)
                flush(prev)

                # copy accumulators out of PSUM first so the banks free early
                avs = sp.tile([128, 512], F32, tag="avs", bufs=6, name=f"avs{qb}_{p}")
                nc.vector.tensor_copy(avs, av)
                rec = sp.tile([128, 512], F32, tag="rec", bufs=6, name=f"rec{qb}_{p}")
                nc.vector.reciprocal_approx_fast(rec, den)
                nc.vector.tensor_mul(AT[p][:, q0:q0 + 512], avs, rec)

            # ------------- emission: weave QKV(p), early attention, outproj -------------
            # Pairs 0/1 projected first; their attention units then interleave with
            # the pair-2/3 projections so ScalarE exps overlap PE-bound QKV work.
            weave = [("q", 0, 0), ("q", 0, 1), ("q", 0, 2), ("u", 0, 0),
                     ("q", 1, 0), ("q", 1, 1), ("q", 1, 2), ("u", 0, 1),
                     ("q", 2, 0), ("u", 1, 0), ("q", 2, 1), ("u", 1, 1),
                     ("q", 2, 2), ("u", 2, 0), ("q", 3, 0), ("u", 2, 1),
                     ("q", 3, 1), ("u", 3, 0), ("q", 3, 2)]
            for kind, a, b in weave:
                if kind == "u":
                    attn_unit(a, b)
                else:
                    qkv_mtile(a, b)
            attn_unit(0, 2)
            attn_unit(0, 3)
            outproj(0)
            attn_unit(3, 1)
            outproj(1)
            attn_unit(1, 2)
            outproj(2)
            attn_unit(1, 3)
            outproj(3)
            outproj(4)
            attn_unit(2, 2)
            outproj(5)
            attn_unit(2, 3)
            outproj(6)
            outproj(7)
            attn_unit(3, 2)
            outproj(8)
            outproj(9)
            outproj(10)
            attn_unit(3, 3)
            outproj(11)
            for tt in (12, 13, 14, 15):
                outproj(tt)

    nc.finalize()
    return nc


_NC = None


def kernel(x, qkv_w, qkv_b, out_w, out_b, attn_mask):
    global _NC, LAST_RESULT
    if _NC is None:
        _NC = build()

    x = np.asarray(x, dtype=np.float32)
    qkv_w = np.asarray(qkv_w, dtype=np.float32)
    qkv_b = np.asarray(qkv_b, dtype=np.float32)
    out_w = np.asarray(out_w, dtype=np.float32)
    out_b = np.asarray(out_b, dtype=np.float32)
    mask = np.asarray(attn_mask).reshape(T, T)

    # mask^T band tiles: band[:, j, :] = mask[qb*512:(qb+1)*512, j*128:(j+1)*128].T
    band = np.empty((128, NKT, 512), dtype=ml_dtypes.bfloat16)
    for j in range(NKT):
        qb = j // 4
        band[:, j, :] = mask[qb * 512:(qb + 1) * 512, j * 128:(j + 1) * 128].astype(
            ml_dtypes.bfloat16).T

    in_maps = []
    for c in range(8):
        b, hg = c % 4, c // 4
        h0 = hg * HPC
        # per-pair [q;k;v] row blocks of qkv_w, transposed; q pre-scaled by 1/8
        blocks = []
        bias_cols = np.empty((128, MT), dtype=np.float32)
        for p in range(PAIRS):
            r0 = (h0 + 2 * p) * D
            qrows = qkv_w[r0:r0 + 128] * 0.125
            krows = qkv_w[C + r0:C + r0 + 128]
            vrows = qkv_w[2 * C + r0:2 * C + r0 + 128]
            blocks += [qrows, krows, vrows]
            bias_cols[:, 3 * p + 0] = qkv_b[r0:r0 + 128] * 0.125
            bias_cols[:, 3 * p + 1] = qkv_b[C + r0:C + r0 + 128]
            bias_cols[:, 3 * p + 2] = qkv_b[2 * C + r0:2 * C + r0 + 128]
        wqkvT = np.ascontiguousarray(np.concatenate(blocks, axis=0).T).astype(ml_dtypes.bfloat16)
        woT = np.ascontiguousarray(
            out_w[:, h0 * D:(h0 + HPC) * D].T).astype(ml_dtypes.bfloat16)
        bo = (np.tile(out_b, (128, 1)) if hg == 0
              else np.zeros((128, C), np.float32)).astype(np.float32)
        in_maps.append({
            "xT": np.ascontiguousarray(x[b].T).astype(ml_dtypes.bfloat16),
            "wqkvT": wqkvT,
            "bqkv": bias_cols,
            "woT": woT,
            "bo": bo,
            "maskband": band,
        })

    LAST_RESULT = run_bass_kernel_spmd(_NC, in_maps, core_ids=list(range(8)))
    res = LAST_RESULT.results
    out = np.empty((B, T, C), dtype=np.float32)
    for b in range(B):
        out[b] = res[b]["out"] + res[b + 4]["out"]
    return out


# revision 35
# speedup vs baseline: 1.0072x; 1.0072x over previous
"""Multi-head causal attention (B=4, T=2048, C=1024, H=16) on 8 TRN2 NeuronCores.

Sharding: data-parallel over batch (4) x tensor-parallel over heads (2 groups
of 8 heads). Core c handles batch c%4, head-group c//4. Each core:
  - QKV projection in transposed layout (float32r matmuls, full PE rate):
    Q^T/K^T/V^T [m, t] tiles, bias folded into the psum->SBUF copy (bf16 out).
  - V^T -> V via PE transposes.
  - Causal flash-style attention per head-pair (2 heads packed per matmul via
    row/col tile_position): S^T = K^T.T @ Q^T, E = exp(S^T) on ScalarE,
    input mask applied on diagonal-band tiles, AV and denominator (ones-matmul)
    accumulated over key tiles; no max-subtraction needed (logits are small:
    softmax is shift-invariant, fp32 exp cannot overflow here).
  - Row-parallel output projection producing a partial [T, C] sum; host adds
    the two head-group partials (out bias is added on head-group-0 cores only).
"""

import os
import sys

sys.path.insert(0, "/opt/trn_rl_repo")

import numpy as np
import ml_dtypes

import concourse.bacc as bacc
import concourse.tile as tile
from concourse import mybir
from concourse.bass_utils import run_bass_kernel_spmd
from concourse.masks import make_identity

B, T, C, H, D = 4, 2048, 1024, 16, 64
HPC = 8          # heads per core
PAIRS = HPC // 2
CT = C // 128    # 8 contraction tiles for the projections
MT = 12          # qkv m-tiles per core (4 pairs x {q,k,v})
NQB = T // 512   # 4 query blocks of 512
NKT = T // 128   # 16 key tiles of 128

F32 = mybir.dt.float32
F32R = mybir.dt.float32r
BF16 = mybir.dt.bfloat16

LAST_RESULT = None  # stashed BassKernelResults for test harnesses


def build():
    nc = bacc.Bacc("TRN2", target_bir_lowering=False)

    xT = nc.dram_tensor("xT", [C, T], BF16, kind="ExternalInput")
    wqkvT = nc.dram_tensor("wqkvT", [C, MT * 128], BF16, kind="ExternalInput")
    bqkv = nc.dram_tensor("bqkv", [128, MT], F32, kind="ExternalInput")
    woT = nc.dram_tensor("woT", [512, C], BF16, kind="ExternalInput")
    bo = nc.dram_tensor("bo", [128, C], F32, kind="ExternalInput")
    maskband = nc.dram_tensor("maskband", [128, NKT, 512], BF16, kind="ExternalInput")
    out = nc.dram_tensor("out", [T, C], F32, kind="ExternalOutput")

    with tile.TileContext(nc) as tc:
        with tc.tile_pool(name="persist", bufs=1) as pp, \
             tc.tile_pool(name="stream", bufs=2) as sp, \
             tc.tile_pool(name="pss", bufs=3, space="PSUM") as pss, \
             tc.tile_pool(name="psav", bufs=1, space="PSUM") as psav, \
             tc.tile_pool(name="psden", bufs=1, space="PSUM") as psden:

            # -------- weight DMAs for the first m-tiles go out first --------
            wm_tiles = {}

            def fetch_wm(mt):
                wm = sp.tile([128, CT, 128], BF16, tag="wm", bufs=3, name=f"wm{mt}")
                nc.sync.dma_start(
                    out=wm,
                    in_=wqkvT[:, mt * 128:(mt + 1) * 128]
                    .rearrange("(n p) m -> p n m", p=128))
                wm_tiles[mt] = wm

            fetch_wm(0)
            xt_sb = []
            for ct in range(CT):
                t_ = pp.tile([128, T], BF16, tag="xt", bufs=CT, name=f"xt{ct}")
                # quarter DMA first so the first matmuls can start early
                nc.sync.dma_start(out=t_[:, 0:512],
                                  in_=xT[ct * 128:(ct + 1) * 128, 0:512])
                xt_sb.append(t_)
            fetch_wm(1)
            for ct in range(CT):
                nc.sync.dma_start(out=xt_sb[ct][:, 512:1024],
                                  in_=xT[ct * 128:(ct + 1) * 128, 512:1024])
            fetch_wm(2)
            for ct in range(CT):
                nc.sync.dma_start(out=xt_sb[ct][:, 1024:2048],
                                  in_=xT[ct * 128:(ct + 1) * 128, 1024:2048])

            # ---------------- other constants / persistent inputs ----------------
            ones_sb = pp.tile([128, 64], BF16, tag="ones", name="ones")
            nc.vector.memset(ones_sb, 1.0)
            ident = pp.tile([128, 128], BF16, tag="ident", name="ident")
            make_identity(nc, ident)
            bqkv_sb = pp.tile([128, MT], F32, tag="bqkv", name="bqkv_sb")
            nc.sync.dma_start(out=bqkv_sb, in_=bqkv[:, :])
            bo_sb = pp.tile([128, C], F32, tag="bo", name="bo_sb")
            nc.sync.dma_start(out=bo_sb, in_=bo[:, :])
            mask_sb = pp.tile([128, NKT, 512], BF16, tag="mask", name="mask_sb")
            nc.sync.dma_start(out=mask_sb, in_=maskband[:, :, :])
            wo_sb = []
            for it in range(4):
                w = pp.tile([128, C], BF16, tag="wo", bufs=4, name=f"wo{it}")
                nc.sync.dma_start(out=w, in_=woT[it * 128:(it + 1) * 128, :])
                wo_sb.append(w)

            QT = [pp.tile([128, T], BF16, tag="qt", bufs=4, name=f"qt{p}") for p in range(4)]
            KT = [pp.tile([128, T], BF16, tag="kt", bufs=4, name=f"kt{p}") for p in range(4)]
            V = [pp.tile([128, T], BF16, tag="v", bufs=4, name=f"v{p}") for p in range(4)]
            AT = [pp.tile([128, T], BF16, tag="at", bufs=4, name=f"at{p}") for p in range(4)]

            # ---------------- QKV projection (transposed output) ----------------
            def qkv_mtile(p, j):
                mt = p * 3 + j
                if mt + 1 < MT and (mt + 1) not in wm_tiles:
                    fetch_wm(mt + 1)   # prefetch next weight slice
                wm = wm_tiles.pop(mt)
                dst = (QT[p], KT[p], None)[j]
                if j == 2:
                    dst = pp.tile([128, T], BF16, tag="vt", bufs=2, name=f"vt{p}")
                if mt == 0:  # fine-grained start using the idle av/den banks
                    for tch in range(4):
                        pool = (psav, psden)[tch % 2]
                        ps = pool.tile([128, 512], F32, tag=("av", "den")[tch % 2],
                                       name=f"qkv_ps0_{tch}")
                        for ct in range(CT):
                            nc.tensor.matmul(
                                ps, wm[:, ct, :], xt_sb[ct][:, tch * 512:(tch + 1) * 512],
                                start=(ct == 0), stop=(ct == CT - 1))
                        nc.vector.tensor_scalar_add(
                            dst[:, tch * 512:(tch + 1) * 512], ps, bqkv_sb[:, mt:mt + 1])
                    return
                for tch in range(2):
                    ps = pss.tile([128, 1024], F32, tag="pss", name=f"qkv_ps{mt}_{tch}")
                    for ct in range(CT):
                        for hf in range(2):
                            t0 = tch * 1024 + hf * 512
                            nc.tensor.matmul(
                                ps[:, hf * 512:(hf + 1) * 512],
                                wm[:, ct, :], xt_sb[ct][:, t0:t0 + 512],
                                start=(ct == 0), stop=(ct == CT - 1))
                    nc.vector.tensor_scalar_add(
                        dst[:, tch * 1024:(tch + 1) * 1024], ps, bqkv_sb[:, mt:mt + 1])
                if j == 2:  # V^T -> V natural layout via PE transposes
                    for g in range(4):
                        pst = psav.tile([128, 512], BF16, tag="av", name=f"vtr{p}_{g}")
                        for c4 in range(4):
                            k = g * 4 + c4
                            nc.tensor.transpose(
                                pst[:, c4 * 128:(c4 + 1) * 128],
                                dst[:, k * 128:(k + 1) * 128], ident)
                        nc.vector.tensor_copy(V[p][:, g * 512:(g + 1) * 512], pst)

            # ---------------- attention unit + out-projection ----------------
            def outproj(tt):
                po = pss.tile([128, 1024], F32, tag="pss", name=f"op{tt}")
                for oc in range(2):
                    for it in range(4):
                        nc.tensor.matmul(
                            po[:, oc * 512:(oc + 1) * 512],
                            AT[it][:, tt * 128:(tt + 1) * 128],
                            wo_sb[it][:, oc * 512:(oc + 1) * 512],
                            start=(it == 0), stop=(it == 3))
                o = sp.tile([128, 1024], F32, tag="o", bufs=3, name=f"o{tt}")
                nc.vector.tensor_add(o, po, bo_sb)
                nc.sync.dma_start(out=out[tt * 128:(tt + 1) * 128, :], in_=o)

            def attn_unit(qb, p):
                nkt = qb * 4 + 4
                q0 = qb * 512
                av = psav.tile([128, 512], F32, tag="av", name=f"av{qb}_{p}")
                den = psden.tile([128, 512], F32, tag="den", name=f"den{qb}_{p}")

                def flush(prev):
                    k, e, off, w = prev
                    st, sp_ = (k == 0), (k == nkt - 1)
                    nc.tensor.matmul(av[0:64, off:off + w],
                                     V[p][:, k * 128:k * 128 + 64], e[:, 0:w],
                                     start=st, stop=sp_, skip_group_check=True)
                    nc.tensor.matmul(av[64:128, off:off + w],
                                     V[p][:, k * 128 + 64:(k + 1) * 128], e[:, 512:512 + w],
                                     start=st, stop=sp_, skip_group_check=True)
                    nc.tensor.matmul(den[0:64, off:off + w], ones_sb, e[:, 0:w],
                                     start=st, stop=sp_, skip_group_check=True)
                    nc.tensor.matmul(den[64:128, off:off + w], ones_sb, e[:, 512:512 + w],
                                     start=st, stop=sp_, skip_group_check=True)

                prev = None
                for k in range(nkt):
                    koff = k - qb * 4
                    # causal: kj-tile k only reaches queries qi >= k*128
                    off = max(koff, 0) * 128
                    w = 512 - off
                    qa = q0 + off
                    ss = pss.tile([128, 1024], F32, tag="pss", name=f"ss{qb}_{p}_{k}")
                    nc.tensor.matmul(ss[:, 0:w], KT[p][0:64, k * 128:(k + 1) * 128],
                                     QT[p][0:64, qa:qa + w], start=True, stop=True)
                    nc.tensor.matmul(ss[:, 512:512 + w], KT[p][64:128, k * 128:(k + 1) * 128],
                                     QT[p][64:128, qa:qa + w], start=True, stop=True)
                    e = sp.tile([128, 2, 512], BF16, tag="e", bufs=10, name=f"e{qb}_{p}_{k}")
                    nc.scalar.activation(
                        e[:, :, 0:w],
                        ss[:, :].rearrange("a (two n) -> a two n", two=2)[:, :, 0:w],
                        mybir.ActivationFunctionType.Exp)
                    ef = e.rearrange("a two n -> a (two n)")
                    if koff >= 0:
                        # apply the input mask on the leading 128-wide block only:
                        # beyond it every query index exceeds all keys of this tile
                        # (causal tril), so the mask there is all-ones
                        nc.vector.tensor_mul(ef[:, 0:128], ef[:, 0:128],
                                             mask_sb[:, k, off:off + 128])
                        nc.vector.tensor_mul(ef[:, 512:640], ef[:, 512:640],
                                             mask_sb[:, k, off:off + 128])
                    if prev is not None:
                        flush(prev)
                    prev = (k, ef, off, w)
                flush(prev)

                # copy accumulators out of PSUM first so the banks free early
                avs = sp.tile([128, 512], F32, tag="avs", bufs=4, name=f"avs{qb}_{p}")
                nc.vector.tensor_copy(avs, av)
                rec = sp.tile([128, 512], F32, tag="rec", bufs=6, name=f"rec{qb}_{p}")
                nc.vector.reciprocal_approx_fast(rec, den)
                nc.vector.tensor_mul(AT[p][:, q0:q0 + 512], avs, rec)

            # ------------- emission: weave QKV(p), early attention, outproj -------------
            # Pairs 0/1 projected first; their attention units then interleave with
            # the pair-2/3 projections so ScalarE exps overlap PE-bound QKV work.
            weave = [("q", 0, 0), ("q", 0, 1), ("q", 0, 2), ("u", 0, 0),
                     ("q", 1, 0), ("q", 1, 1), ("q", 1, 2), ("u", 0, 1),
                     ("q", 2, 0), ("u", 1, 0), ("q", 2, 1), ("u", 1, 1),
                     ("q", 2, 2), ("u", 2, 0), ("q", 3, 0), ("u", 2, 1),
                     ("q", 3, 1), ("u", 3, 0), ("q", 3, 2)]
            for kind, a, b in weave:
                if kind == "u":
                    attn_unit(a, b)
                else:
                    qkv_mtile(a, b)
            attn_unit(0, 2)
            attn_unit(0, 3)
            outproj(0)
            attn_unit(3, 1)
            outproj(1)
            attn_unit(1, 2)
            outproj(2)
            attn_unit(1, 3)
            outproj(3)
            outproj(4)
            attn_unit(2, 2)
            outproj(5)
            attn_unit(2, 3)
            outproj(6)
            outproj(7)
            attn_unit(3, 2)
            outproj(8)
            outproj(9)
            outproj(10)
            attn_unit(3, 3)
            outproj(11)
            for tt in (12, 13, 14, 15):
                outproj(tt)

    nc.finalize()
    return nc


_NC = None


def kernel(x, qkv_w, qkv_b, out_w, out_b, attn_mask):
    global _NC, LAST_RESULT
    if _NC is None:
        _NC = build()

    x = np.asarray(x, dtype=np.float32)
    qkv_w = np.asarray(qkv_w, dtype=np.float32)
    qkv_b = np.asarray(qkv_b, dtype=np.float32)
    out_w = np.asarray(out_w, dtype=np.float32)
    out_b = np.asarray(out_b, dtype=np.float32)
    mask = np.asarray(attn_mask).reshape(T, T)

    # mask^T band tiles: band[:, j, :] = mask[qb*512:(qb+1)*512, j*128:(j+1)*128].T
    band = np.empty((128, NKT, 512), dtype=ml_dtypes.bfloat16)
    for j in range(NKT):
        qb = j // 4
        band[:, j, :] = mask[qb * 512:(qb + 1) * 512, j * 128:(j + 1) * 128].astype(
            ml_dtypes.bfloat16).T

    in_maps = []
    for c in range(8):
        b, hg = c % 4, c // 4
        h0 = hg * HPC
        # per-pair [q;k;v] row blocks of qkv_w, transposed; q pre-scaled by 1/8
        blocks = []
        bias_cols = np.empty((128, MT), dtype=np.float32)
        for p in range(PAIRS):
            r0 = (h0 + 2 * p) * D
            qrows = qkv_w[r0:r0 + 128] * 0.125
            krows = qkv_w[C + r0:C + r0 + 128]
            vrows = qkv_w[2 * C + r0:2 * C + r0 + 128]
            blocks += [qrows, krows, vrows]
            bias_cols[:, 3 * p + 0] = qkv_b[r0:r0 + 128] * 0.125
            bias_cols[:, 3 * p + 1] = qkv_b[C + r0:C + r0 + 128]
            bias_cols[:, 3 * p + 2] = qkv_b[2 * C + r0:2 * C + r0 + 128]
        wqkvT = np.ascontiguousarray(np.concatenate(blocks, axis=0).T).astype(ml_dtypes.bfloat16)
        woT = np.ascontiguousarray(
            out_w[:, h0 * D:(h0 + HPC) * D].T).astype(ml_dtypes.bfloat16)
        bo = (np.tile(out_b, (128, 1)) if hg == 0
              else np.zeros((128, C), np.float32)).astype(np.float32)
        in_maps.append({
            "xT": np.ascontiguousarray(x[b].T).astype(ml_dtypes.bfloat16),
            "wqkvT": wqkvT,
            "bqkv": bias_cols,
            "woT": woT,
            "bo": bo,
            "maskband": band,
        })

    LAST_RESULT = run_bass_kernel_spmd(_NC, in_maps, core_ids=list(range(8)))
    res = LAST_RESULT.results
    out = np.empty((B, T, C), dtype=np.float32)
    for b in range(B):
        out[b] = res[b]["out"] + res[b + 4]["out"]
    return out
